# revision 3
# baseline (speedup 1.0000x reference)
"""Trainium2 Bass kernel for nn_Embedding_loss (masked per-instance embedding loss).

Math: for each instance k with class c_k, over the (H,W) plane:
    cnt_k = sum(mask_k), s1_k = sum(emb[c_k] * mask_k), s2_k = sum(emb[c_k]^2 * mask_k)
With m1 = emb * mask and mask in {0,1}:  s2_k = sum(m1^2).
Per-instance means/variances plus the tiny O(K^2) pairwise hinge term are
assembled on the host from the (s1, s2, cnt) triples.

Sharding: K instances are split across 8 cores (13 per core, zero-padded).
The host stages each instance's class plane and mask interleaved as one fp8
tensor [P, KPC, 2, F] (0/1 exact for masks; fp8 quantization of the
embeddings moves the final loss by ~2e-5 relative) and counts mask bits
host-side while staging.

Device pipeline per core — work is spread over three engines so no single
engine serializes the 26 elementwise passes (13 multiplies + 13 squares):
  - 8 instances: VectorE STT m1 = plane*mask (fp8 1x, fp16 out), accum -> s1
  - 5 instances: GPSIMD tensor_tensor m1 = plane*mask; s1 recovered on
    VectorE via fp16 tensor_scalar (4x mode) with accum
  - squares: 10 on ScalarE (Square activation + accum), 3 on VectorE as
    fp16 STT m1*m1 (2x mode) with accum
DMAs are issued in 4 waves ([2,3,4,4] instances) to overlap load/compute
with few triggers.
"""

import os

import numpy as np

import concourse.bass as bass
import concourse.tile as tile
from concourse import mybir
from concourse.bass_utils import run_bass_kernel_spmd

N_CORES = 8
C, H, W = 80, 512, 512
K = 100
KPC = 13  # instances per core (8*13 = 104 >= 100, padded with zero masks)
P = 128  # SBUF partitions
F = (H * W) // P  # free-dim elements per partition (2048)

WAVES = [2, 3, 4, 4]  # instances per DMA wave
GPS_M = {4, 7, 8, 11, 12}  # multiply on GPSIMD (rest on VectorE)
DVE_SQ = {3, 6, 10}  # square+accum on VectorE (rest on ScalarE)

_NC_CACHE = None
LAST_RESULT = None  # BassKernelResults of the most recent run (for test harness)


def _split_sync(nc, max_w=1, max_u=1):
    """Walrus in this env accepts at most one sync wait/update per instruction;
    Tile's kernel-tail drain aggregates several. Split extras onto NoOps on the
    same engine (sequential waits on one queue are an AND, so semantics hold)."""
    ctr = 0
    for f in nc.m.functions:
        for bb in f.blocks:
            new = []
            for inst in bb.instructions:
                si = getattr(inst, "sync_info", None)
                waits = list(si.on_wait) if si is not None and si.on_wait else []
                updates = (
                    list(si.on_update) if si is not None and si.on_update else []
                )
                pre, post = [], []
                if len(waits) > max_w:
                    extra, keep = waits[:-max_w], waits[-max_w:]
                    si.on_wait = keep
                    for w in extra:
                        ctr += 1
                        nop = mybir.InstNoOp(name=f"syncsplit-w-{ctr}", ins=[], outs=[])
                        nop.engine = inst.engine
                        nop.sync_info = mybir.SyncInfo(on_wait=[w], on_update=[])
                        pre.append(nop)
                if len(updates) > max_u:
                    keep_u, extra_u = updates[:max_u], updates[max_u:]
                    si.on_update = keep_u
                    for u in extra_u:
                        ctr += 1
                        nop = mybir.InstNoOp(name=f"syncsplit-u-{ctr}", ins=[], outs=[])
                        nop.engine = inst.engine
                        nop.sync_info = mybir.SyncInfo(on_wait=[], on_update=[u])
                        post.append(nop)
                new.extend(pre)
                new.append(inst)
                new.extend(post)
            bb.instructions = new


def _build_program():
    """One SPMD Bass program: stream KPC (plane, mask) pairs, emit (s1, s2)."""
    global _NC_CACHE
    if _NC_CACHE is not None:
        return _NC_CACHE

    nc = bass.Bass()
    data = nc.declare_dram_parameter(
        "data", [P, KPC, 2, F], mybir.dt.float8e4, isOutput=False
    )
    # stats columns: [0:KPC) = s1 partials, [KPC:2*KPC) = s2 partials
    stats = nc.declare_dram_parameter(
        "stats", [P, 2 * KPC], mybir.dt.float32, isOutput=True
    )

    waves = []
    lo = 0
    for w in WAVES:
        waves.append((lo, lo + w))
        lo += w
    assert lo == KPC

    with tile.TileContext(nc) as tc:
        with (
            tc.tile_pool(name="io", bufs=2) as io,
            tc.tile_pool(name="m1p", bufs=6) as m1p,
            tc.tile_pool(name="junkp", bufs=4) as junkp,
            tc.tile_pool(name="statp", bufs=1) as statp,
        ):
            st = statp.tile([P, 2 * KPC], mybir.dt.float32)
            for lo, hi in waves:
                n = hi - lo
                dg = io.tile([P, max(WAVES), 2, F], mybir.dt.float8e4, tag="d")
                nc.sync.dma_start(out=dg[:, :n, :, :], in_=data[:, lo:hi, :, :])

                insts = list(range(lo, hi))
                m1s = {}
                # GPSIMD multiplies first: longest pole on the Pool queue
                for j in insts:
                    i = j - lo
                    if j in GPS_M:
                        m1 = m1p.tile([P, F], mybir.dt.float16, tag="m1")
                        nc.gpsimd.tensor_tensor(
                            out=m1,
                            in0=dg[:, i, 0, :],
                            in1=dg[:, i, 1, :],
                            op=mybir.AluOpType.mult,
                        )
                        m1s[j] = m1
                # VectorE multiplies with fused s1 accumulation
                for j in insts:
                    i = j - lo
                    if j not in GPS_M:
                        m1 = m1p.tile([P, F], mybir.dt.float16, tag="m1")
                        nc.vector.scalar_tensor_tensor(
                            out=m1,
                            in0=dg[:, i, 0, :],
                            scalar=1.0,
                            in1=dg[:, i, 1, :],
                            op0=mybir.AluOpType.mult,
                            op1=mybir.AluOpType.mult,
                            accum_out=st[:, j : j + 1],
                        )
                        m1s[j] = m1
                # s1 for GPSIMD instances: fp16 tensor_scalar (4x) + accum
                for j in insts:
                    if j in GPS_M:
                        junk = junkp.tile([P, F], mybir.dt.float16, tag="junk")
                        nc.vector.tensor_scalar(
                            out=junk,
                            in0=m1s[j],
                            scalar1=1.0,
                            scalar2=0.0,
                            op0=mybir.AluOpType.mult,
                            op1=mybir.AluOpType.add,
                            accum_out=st[:, j : j + 1],
                        )
                # squares: s2 partials = sum(m1^2)
                for j in insts:
                    junk = junkp.tile([P, F], mybir.dt.float16, tag="junk")
                    if j in DVE_SQ:
                        nc.vector.scalar_tensor_tensor(
                            out=junk,
                            in0=m1s[j],
                            scalar=1.0,
                            in1=m1s[j],
                            op0=mybir.AluOpType.mult,
                            op1=mybir.AluOpType.mult,
                            accum_out=st[:, KPC + j : KPC + j + 1],
                        )
                    else:
                        nc.scalar.activation(
                            out=junk,
                            in_=m1s[j],
                            func=mybir.ActivationFunctionType.Square,
                            accum_out=st[:, KPC + j : KPC + j + 1],
                        )

            nc.sync.dma_start(out=stats[:, :], in_=st)

    _NC_CACHE = nc
    return nc


def _enable_jax_compile_cache():
    try:
        import jax

        jax.config.update("jax_compilation_cache_dir", "/tmp/jax_neff_cache")
        jax.config.update("jax_persistent_cache_min_entry_size_bytes", -1)
        jax.config.update("jax_persistent_cache_min_compile_time_secs", 0.0)
    except Exception:
        pass
    # NEFF disk cache keyed on BIR bytes (deterministic serialization):
    # skip walrus recompiles across processes.
    try:
        import hashlib
        import shutil

        from concourse import bass2jax

        orig = bass2jax.compile_bir_kernel
        if getattr(orig, "_neff_cache_wrapped", False):
            return

        def cached_compile(bir_json, tmpdir, neff_name="file.neff"):
            h = hashlib.sha256(
                bir_json if isinstance(bir_json, bytes) else bir_json.encode()
            ).hexdigest()
            cpath = f"/tmp/neff_cache/{h}.neff"
            if os.path.exists(cpath):
                dst = os.path.join(tmpdir, neff_name)
                shutil.copy(cpath, dst)
                return dst
            out = orig(bir_json, tmpdir, neff_name=neff_name)
            os.makedirs("/tmp/neff_cache", exist_ok=True)
            shutil.copy(out, cpath)
            return out

        cached_compile._neff_cache_wrapped = True
        bass2jax.compile_bir_kernel = cached_compile
    except Exception:
        pass


def kernel(pred_emb, gt_objmask, gt_classes):
    global LAST_RESULT
    pred_emb = np.asarray(pred_emb)
    gt_objmask = np.asarray(gt_objmask)
    cls = np.clip(np.asarray(gt_classes).astype(np.int64), 0, C - 1)
    k = gt_objmask.shape[0]

    _enable_jax_compile_cache()
    nc = _build_program()
    if not getattr(nc, "_sync_split_done", False):
        _split_sync(nc)  # CoreSim can't execute the bare NoOps; HW path only
        nc._sync_split_done = True

    f8 = mybir.dt.np(mybir.dt.float8e4)
    emb8 = pred_emb.astype(f8).reshape(C, P, F)
    one_f8 = np.ones((), dtype=f8).view(np.uint8)  # bit pattern of fp8 1.0
    mask8 = (gt_objmask.astype(np.uint8) * one_f8).view(f8).reshape(k, P, F)
    cnt = np.count_nonzero(gt_objmask.reshape(k, -1), axis=1).astype(np.float64)

    in_maps = []
    for c in range(N_CORES):
        lo, hi = c * KPC, min((c + 1) * KPC, k)
        n = max(hi - lo, 0)
        dat = np.zeros((P, KPC, 2, F), dtype=f8)
        if n > 0:
            dat[:, :n, 0] = emb8[cls[lo:hi]].transpose(1, 0, 2)
            dat[:, :n, 1] = mask8[lo:hi].transpose(1, 0, 2)
        in_maps.append({"data": dat})

    core_ids = list(range(N_CORES))
    trace = bool(os.environ.get("KERNEL_TRACE"))
    res = run_bass_kernel_spmd(
        nc,
        in_maps,
        core_ids,
        trace=trace,
        trace_cores=core_ids if trace else None,
    )
    LAST_RESULT = res

    s1 = np.zeros(k, dtype=np.float64)
    s2 = np.zeros(k, dtype=np.float64)
    for c in range(N_CORES):
        lo, hi = c * KPC, min((c + 1) * KPC, k)
        n = max(hi - lo, 0)
        if n == 0:
            continue
        stats = res.results[c]["stats"].astype(np.float64)  # (P, 2*KPC)
        s1[lo:hi] = stats[:, 0:KPC].sum(axis=0)[:n]
        s2[lo:hi] = stats[:, KPC : 2 * KPC].sum(axis=0)[:n]

    has = cnt > 0
    safe = np.where(has, cnt, 1.0)
    mean = np.where(has, s1 / safe, 0.0)
    var = np.where(has, s2 / safe - mean * mean, 0.0)

    same = cls[:, None] == cls[None, :]
    upper = np.triu(np.ones((k, k), dtype=bool), 1)
    diff2 = (mean[:, None] - mean[None, :]) ** 2
    hinge = np.maximum(1.0 - diff2, 0.0)
    loss_inter = np.sum(np.where(same & upper, hinge, 0.0))
    loss_reg = np.mean(mean * mean)
    loss_intra = np.mean(var)
    loss = 1.0 * loss_inter + 1.0 * loss_reg + 1.0 * loss_intra
    return np.array([loss], dtype=np.float32)


# revision 4
# speedup vs baseline: 1.0950x; 1.0950x over previous
"""Trainium2 Bass kernel for nn_Embedding_loss (masked per-instance embedding loss).

Math: for each instance k with class c_k, over the (H,W) plane:
    cnt_k = sum(mask_k), s1_k = sum(emb[c_k] * mask_k), s2_k = sum(emb[c_k]^2 * mask_k)
With m1 = emb * mask and mask in {0,1}:  s2_k = sum(m1^2).
Per-instance means/variances plus the tiny O(K^2) pairwise hinge term are
assembled on the host from the (s1, s2, cnt) triples.

Sharding: K instances are split across 8 cores (13 per core, zero-padded).
The host stages each instance's class plane and mask interleaved as one fp8
tensor [P, KPC, 2, F] (0/1 exact for masks; fp8 quantization of the
embeddings moves the final loss by ~2e-5 relative) and counts mask bits
host-side while staging.

Device pipeline per core — work is spread over three engines so no single
engine serializes the 26 elementwise passes (13 multiplies + 13 squares):
  - 8 instances: VectorE STT m1 = plane*mask (fp8 1x, fp16 out), accum -> s1
  - 5 instances: GPSIMD tensor_tensor m1 = plane*mask; s1 recovered on
    VectorE via fp16 tensor_scalar (4x mode) with accum
  - squares: 10 on ScalarE (Square activation + accum), 3 on VectorE as
    fp16 STT m1*m1 (2x mode) with accum
DMAs are issued in 4 waves ([2,3,4,4] instances) to overlap load/compute
with few triggers.
"""

import os

import numpy as np

import concourse.bass as bass
import concourse.tile as tile
from concourse import mybir
from concourse.bass_utils import run_bass_kernel_spmd

N_CORES = 8
C, H, W = 80, 512, 512
K = 100
KPC = 13  # instances per core (8*13 = 104 >= 100, padded with zero masks)
P = 128  # SBUF partitions
F = (H * W) // P  # free-dim elements per partition (2048)

WAVES = [2, 3, 4, 4]  # instances per DMA wave
GPS_M = set()  # GPSIMD compute contends with DVE's SBUF port: net loss. Keep empty.
DVE_SQ = {5, 9}  # square+accum on VectorE (rest on ScalarE)

_NC_CACHE = None
LAST_RESULT = None  # BassKernelResults of the most recent run (for test harness)


def _split_sync(nc, max_w=1, max_u=1):
    """Walrus in this env accepts at most one sync wait/update per instruction;
    Tile's kernel-tail drain aggregates several. Split extras onto NoOps on the
    same engine (sequential waits on one queue are an AND, so semantics hold)."""
    ctr = 0
    for f in nc.m.functions:
        for bb in f.blocks:
            new = []
            for inst in bb.instructions:
                si = getattr(inst, "sync_info", None)
                waits = list(si.on_wait) if si is not None and si.on_wait else []
                updates = (
                    list(si.on_update) if si is not None and si.on_update else []
                )
                pre, post = [], []
                if len(waits) > max_w:
                    extra, keep = waits[:-max_w], waits[-max_w:]
                    si.on_wait = keep
                    for w in extra:
                        ctr += 1
                        nop = mybir.InstNoOp(name=f"syncsplit-w-{ctr}", ins=[], outs=[])
                        nop.engine = inst.engine
                        nop.sync_info = mybir.SyncInfo(on_wait=[w], on_update=[])
                        pre.append(nop)
                if len(updates) > max_u:
                    keep_u, extra_u = updates[:max_u], updates[max_u:]
                    si.on_update = keep_u
                    for u in extra_u:
                        ctr += 1
                        nop = mybir.InstNoOp(name=f"syncsplit-u-{ctr}", ins=[], outs=[])
                        nop.engine = inst.engine
                        nop.sync_info = mybir.SyncInfo(on_wait=[], on_update=[u])
                        post.append(nop)
                new.extend(pre)
                new.append(inst)
                new.extend(post)
            bb.instructions = new


def _build_program():
    """One SPMD Bass program: stream KPC (plane, mask) pairs, emit (s1, s2)."""
    global _NC_CACHE
    if _NC_CACHE is not None:
        return _NC_CACHE

    nc = bass.Bass()
    data = nc.declare_dram_parameter(
        "data", [P, KPC, 2, F], mybir.dt.float8e4, isOutput=False
    )
    # stats columns: [0:KPC) = s1 partials, [KPC:2*KPC) = s2 partials
    stats = nc.declare_dram_parameter(
        "stats", [P, 2 * KPC], mybir.dt.float32, isOutput=True
    )

    waves = []
    lo = 0
    for w in WAVES:
        waves.append((lo, lo + w))
        lo += w
    assert lo == KPC

    with tile.TileContext(nc) as tc:
        with (
            tc.tile_pool(name="io", bufs=2) as io,
            tc.tile_pool(name="m1p", bufs=6) as m1p,
            tc.tile_pool(name="junkp", bufs=4) as junkp,
            tc.tile_pool(name="statp", bufs=1) as statp,
        ):
            st = statp.tile([P, 2 * KPC], mybir.dt.float32)
            for lo, hi in waves:
                n = hi - lo
                dg = io.tile([P, max(WAVES), 2, F], mybir.dt.float8e4, tag="d")
                nc.sync.dma_start(out=dg[:, :n, :, :], in_=data[:, lo:hi, :, :])

                insts = list(range(lo, hi))
                m1s = {}
                # GPSIMD multiplies first: longest pole on the Pool queue
                for j in insts:
                    i = j - lo
                    if j in GPS_M:
                        m1 = m1p.tile([P, F], mybir.dt.float16, tag="m1")
                        nc.gpsimd.tensor_tensor(
                            out=m1,
                            in0=dg[:, i, 0, :],
                            in1=dg[:, i, 1, :],
                            op=mybir.AluOpType.mult,
                        )
                        m1s[j] = m1
                # VectorE multiplies with fused s1 accumulation
                for j in insts:
                    i = j - lo
                    if j not in GPS_M:
                        m1 = m1p.tile([P, F], mybir.dt.float16, tag="m1")
                        nc.vector.scalar_tensor_tensor(
                            out=m1,
                            in0=dg[:, i, 0, :],
                            scalar=1.0,
                            in1=dg[:, i, 1, :],
                            op0=mybir.AluOpType.mult,
                            op1=mybir.AluOpType.mult,
                            accum_out=st[:, j : j + 1],
                        )
                        m1s[j] = m1
                # s1 for GPSIMD instances: fp16 tensor_scalar (4x) + accum
                for j in insts:
                    if j in GPS_M:
                        junk = junkp.tile([P, F], mybir.dt.float16, tag="junk")
                        nc.vector.tensor_scalar(
                            out=junk,
                            in0=m1s[j],
                            scalar1=1.0,
                            scalar2=0.0,
                            op0=mybir.AluOpType.mult,
                            op1=mybir.AluOpType.add,
                            accum_out=st[:, j : j + 1],
                        )
                # squares: s2 partials = sum(m1^2)
                for j in insts:
                    junk = junkp.tile([P, F], mybir.dt.float16, tag="junk")
                    if j in DVE_SQ:
                        nc.vector.scalar_tensor_tensor(
                            out=junk,
                            in0=m1s[j],
                            scalar=1.0,
                            in1=m1s[j],
                            op0=mybir.AluOpType.mult,
                            op1=mybir.AluOpType.mult,
                            accum_out=st[:, KPC + j : KPC + j + 1],
                        )
                    else:
                        nc.scalar.activation(
                            out=junk,
                            in_=m1s[j],
                            func=mybir.ActivationFunctionType.Square,
                            accum_out=st[:, KPC + j : KPC + j + 1],
                        )

            nc.sync.dma_start(out=stats[:, :], in_=st)

    _NC_CACHE = nc
    return nc


def _enable_jax_compile_cache():
    try:
        import jax

        jax.config.update("jax_compilation_cache_dir", "/tmp/jax_neff_cache")
        jax.config.update("jax_persistent_cache_min_entry_size_bytes", -1)
        jax.config.update("jax_persistent_cache_min_compile_time_secs", 0.0)
    except Exception:
        pass
    # NEFF disk cache keyed on BIR bytes (deterministic serialization):
    # skip walrus recompiles across processes.
    try:
        import hashlib
        import shutil

        from concourse import bass2jax

        orig = bass2jax.compile_bir_kernel
        if getattr(orig, "_neff_cache_wrapped", False):
            return

        def cached_compile(bir_json, tmpdir, neff_name="file.neff"):
            h = hashlib.sha256(
                bir_json if isinstance(bir_json, bytes) else bir_json.encode()
            ).hexdigest()
            cpath = f"/tmp/neff_cache/{h}.neff"
            if os.path.exists(cpath):
                dst = os.path.join(tmpdir, neff_name)
                shutil.copy(cpath, dst)
                return dst
            out = orig(bir_json, tmpdir, neff_name=neff_name)
            os.makedirs("/tmp/neff_cache", exist_ok=True)
            shutil.copy(out, cpath)
            return out

        cached_compile._neff_cache_wrapped = True
        bass2jax.compile_bir_kernel = cached_compile
    except Exception:
        pass


def kernel(pred_emb, gt_objmask, gt_classes):
    global LAST_RESULT
    pred_emb = np.asarray(pred_emb)
    gt_objmask = np.asarray(gt_objmask)
    cls = np.clip(np.asarray(gt_classes).astype(np.int64), 0, C - 1)
    k = gt_objmask.shape[0]

    _enable_jax_compile_cache()
    nc = _build_program()
    if not getattr(nc, "_sync_split_done", False):
        _split_sync(nc)  # CoreSim can't execute the bare NoOps; HW path only
        nc._sync_split_done = True

    f8 = mybir.dt.np(mybir.dt.float8e4)
    emb8 = pred_emb.astype(f8).reshape(C, P, F)
    one_f8 = np.ones((), dtype=f8).view(np.uint8)  # bit pattern of fp8 1.0
    mask8 = (gt_objmask.astype(np.uint8) * one_f8).view(f8).reshape(k, P, F)
    cnt = np.count_nonzero(gt_objmask.reshape(k, -1), axis=1).astype(np.float64)

    in_maps = []
    for c in range(N_CORES):
        lo, hi = c * KPC, min((c + 1) * KPC, k)
        n = max(hi - lo, 0)
        dat = np.zeros((P, KPC, 2, F), dtype=f8)
        if n > 0:
            dat[:, :n, 0] = emb8[cls[lo:hi]].transpose(1, 0, 2)
            dat[:, :n, 1] = mask8[lo:hi].transpose(1, 0, 2)
        in_maps.append({"data": dat})

    core_ids = list(range(N_CORES))
    trace = bool(os.environ.get("KERNEL_TRACE"))
    res = run_bass_kernel_spmd(
        nc,
        in_maps,
        core_ids,
        trace=trace,
        trace_cores=core_ids if trace else None,
    )
    LAST_RESULT = res

    s1 = np.zeros(k, dtype=np.float64)
    s2 = np.zeros(k, dtype=np.float64)
    for c in range(N_CORES):
        lo, hi = c * KPC, min((c + 1) * KPC, k)
        n = max(hi - lo, 0)
        if n == 0:
            continue
        stats = res.results[c]["stats"].astype(np.float64)  # (P, 2*KPC)
        s1[lo:hi] = stats[:, 0:KPC].sum(axis=0)[:n]
        s2[lo:hi] = stats[:, KPC : 2 * KPC].sum(axis=0)[:n]

    has = cnt > 0
    safe = np.where(has, cnt, 1.0)
    mean = np.where(has, s1 / safe, 0.0)
    var = np.where(has, s2 / safe - mean * mean, 0.0)

    same = cls[:, None] == cls[None, :]
    upper = np.triu(np.ones((k, k), dtype=bool), 1)
    diff2 = (mean[:, None] - mean[None, :]) ** 2
    hinge = np.maximum(1.0 - diff2, 0.0)
    loss_inter = np.sum(np.where(same & upper, hinge, 0.0))
    loss_reg = np.mean(mean * mean)
    loss_intra = np.mean(var)
    loss = 1.0 * loss_inter + 1.0 * loss_reg + 1.0 * loss_intra
    return np.array([loss], dtype=np.float32)


# revision 6
# speedup vs baseline: 1.1105x; 1.0142x over previous
"""Trainium2 Bass kernel for nn_Embedding_loss (masked per-instance embedding loss).

Math: for each instance k with class c_k, over the (H,W) plane:
    cnt_k = sum(mask_k), s1_k = sum(emb[c_k] * mask_k), s2_k = sum(emb[c_k]^2 * mask_k)
With m1 = emb * mask and mask in {0,1}:  s2_k = sum(m1^2).
Per-instance means/variances plus the tiny O(K^2) pairwise hinge term are
assembled on the host from the (s1, s2, cnt) triples.

Sharding: K instances are split across 8 cores (13 per core, zero-padded).

Device decomposition (per core) — avoids the two 1x-mode walls (fp8 elementwise
and accum_out ops both run at 1 elem/cycle on VectorE):
  - masks staged as 0x00/0xFF bytes; m1 = plane AND mask done on VectorE at
    uint32 granularity (4 fp8 bytes/lane/cycle, exact) in one batched op/wave.
  - squares: one batched ScalarE Square activation per wave (fp8 -> fp16).
  - per-instance sums s1, s2: TensorE matmuls with a ones[128,1] stationary;
    each instance accumulates its 4 chunk-column-sums into its own PSUM row.
    Two VectorE passes (one per PSUM bank) drain all 13 s1 and 13 s2 with a
    single accumulate each.
"""

import os

import numpy as np

import concourse.bass as bass
import concourse.tile as tile
from concourse import mybir
from concourse.bass_utils import run_bass_kernel_spmd

N_CORES = 8
C, H, W = 80, 512, 512
K = 100
KPC = 13  # instances per core (8*13 = 104 >= 100, padded with zero masks)
P = 128  # SBUF partitions
F = (H * W) // P  # free-dim elements per partition (2048)
FW = F // 4  # uint32 words per partition per instance (512)

WAVES = [2, 3, 4, 4]  # instances per DMA wave
ACT_SQ = set(range(KPC))  # squares on ScalarE (batched per wave)
M1_OFF = 196608  # fixed SBUF byte offset of the dual-dtype m1 region

_NC_CACHE = None
LAST_RESULT = None  # BassKernelResults of the most recent run (for test harness)


def _split_sync(nc, max_w=1, max_u=1):
    """Walrus in this env accepts at most one sync wait/update per instruction;
    Tile's kernel-tail drain aggregates several. Split extras onto NoOps on the
    same engine (sequential waits on one queue are an AND, so semantics hold)."""
    ctr = 0
    for f in nc.m.functions:
        for bb in f.blocks:
            new = []
            for inst in bb.instructions:
                si = getattr(inst, "sync_info", None)
                waits = list(si.on_wait) if si is not None and si.on_wait else []
                updates = (
                    list(si.on_update) if si is not None and si.on_update else []
                )
                pre, post = [], []
                if len(waits) > max_w:
                    extra, keep = waits[:-max_w], waits[-max_w:]
                    si.on_wait = keep
                    for w in extra:
                        ctr += 1
                        nop = mybir.InstNoOp(name=f"syncsplit-w-{ctr}", ins=[], outs=[])
                        nop.engine = inst.engine
                        nop.sync_info = mybir.SyncInfo(on_wait=[w], on_update=[])
                        pre.append(nop)
                if len(updates) > max_u:
                    keep_u, extra_u = updates[:max_u], updates[max_u:]
                    si.on_update = keep_u
                    for u in extra_u:
                        ctr += 1
                        nop = mybir.InstNoOp(name=f"syncsplit-u-{ctr}", ins=[], outs=[])
                        nop.engine = inst.engine
                        nop.sync_info = mybir.SyncInfo(on_wait=[], on_update=[u])
                        post.append(nop)
                new.extend(pre)
                new.append(inst)
                new.extend(post)
            bb.instructions = new


def _build_program():
    """One SPMD Bass program: stream KPC (plane, mask) pairs, emit (s1, s2)."""
    global _NC_CACHE
    if _NC_CACHE is not None:
        return _NC_CACHE

    nc = bass.Bass()
    planes = nc.declare_dram_parameter(
        "planes", [P, KPC, FW], mybir.dt.uint32, isOutput=False
    )
    masks = nc.declare_dram_parameter(
        "masks", [P, KPC, FW], mybir.dt.uint32, isOutput=False
    )
    # sliding one-hot: col 13 is 1.0; sel[:, 13-j:26-j] puts the ones-column
    # at relative index j, steering instance j's column sums into PSUM row j.
    sel8 = nc.declare_dram_parameter("sel8", [P, 26], mybir.dt.float8e4, isOutput=False)
    sel16 = nc.declare_dram_parameter(
        "sel16", [P, 26], mybir.dt.float16, isOutput=False
    )
    # stats: col 0 = s1 (rows 0..KPC), col 1 = s2
    stats = nc.declare_dram_parameter("stats", [P, 2], mybir.dt.float32, isOutput=True)

    # m1 region, aliased as uint32 (bitwise-AND dest) and fp8 (Square/matmul src)
    m1u = nc.alloc_sbuf_tensor_at(
        "m1u", [P, KPC, FW], mybir.dt.uint32, offset=M1_OFF
    )
    m1f = nc.alloc_sbuf_tensor_at(
        "m1f", [P, KPC, F], mybir.dt.float8e4, offset=M1_OFF
    )

    waves = []
    lo = 0
    for w in WAVES:
        waves.append((lo, lo + w))
        lo += w
    assert lo == KPC

    with tile.TileContext(nc) as tc:
        with (
            tc.tile_pool(name="io", bufs=2) as io,
            tc.tile_pool(name="sqp", bufs=1) as sqp,
            tc.tile_pool(name="onesp", bufs=1) as onesp,
            tc.tile_pool(name="junkp", bufs=2) as junkp,
            tc.tile_pool(name="statp", bufs=1) as statp,
            tc.tile_pool(name="ps", bufs=1, space="PSUM") as ps,
        ):
            o8 = onesp.tile([P, 26], mybir.dt.float8e4, tag="o8")
            o16 = onesp.tile([P, 26], mybir.dt.float16, tag="o16")
            nc.sync.dma_start(out=o8, in_=sel8[:, :])
            nc.sync.dma_start(out=o16, in_=sel16[:, :])

            st = statp.tile([P, 2], mybir.dt.float32)
            sq = sqp.tile([P, KPC, F], mybir.dt.float16)
            ps1 = ps.tile([KPC, FW], mybir.dt.float32, tag="ps1")
            ps2 = ps.tile([KPC, FW], mybir.dt.float32, tag="ps2")

            for lo, hi in waves:
                n = hi - lo
                pt = io.tile([P, max(WAVES), FW], mybir.dt.uint32, tag="p")
                mt = io.tile([P, max(WAVES), FW], mybir.dt.uint32, tag="m")
                nc.sync.dma_start(out=pt[:, :n, :], in_=planes[:, lo:hi, :])
                nc.sync.dma_start(out=mt[:, :n, :], in_=masks[:, lo:hi, :])

                # batched masked-plane: m1 = plane AND maskFF (exact, 4B/cycle)
                nc.vector.tensor_tensor(
                    out=m1u[:, lo:hi, :],
                    in0=pt[:, :n, :],
                    in1=mt[:, :n, :],
                    op=mybir.AluOpType.bitwise_and,
                )
                # batched squares on ScalarE (fp8 in, fp16 out)
                nc.scalar.activation(
                    out=sq[:, lo:hi, :],
                    in_=m1f[:, lo:hi, :],
                    func=mybir.ActivationFunctionType.Square,
                )
                # per-instance column sums into PSUM rows via ones-stationary
                for j in range(lo, hi):
                    for c in range(4):
                        nc.tensor.matmul(
                            ps1[:, :],
                            o8[:, 13 - j : 26 - j],
                            m1f[:, j, c * FW : (c + 1) * FW],
                            start=(j == 0 and c == 0),
                            stop=(j == KPC - 1 and c == 3),
                            skip_group_check=True,
                        )
                    for c in range(4):
                        nc.tensor.matmul(
                            ps2[:, :],
                            o16[:, 13 - j : 26 - j],
                            sq[:, j, c * FW : (c + 1) * FW],
                            start=(j == 0 and c == 0),
                            stop=(j == KPC - 1 and c == 3),
                            skip_group_check=True,
                        )

            # drain: one accumulate per PSUM bank recovers all KPC sums
            j1 = junkp.tile([KPC, FW], mybir.dt.float32, tag="j1")
            j2 = junkp.tile([KPC, FW], mybir.dt.float32, tag="j2")
            nc.vector.tensor_scalar(
                out=j1,
                in0=ps1,
                scalar1=1.0,
                scalar2=0.0,
                op0=mybir.AluOpType.mult,
                op1=mybir.AluOpType.add,
                accum_out=st[0:KPC, 0:1],
            )
            nc.vector.tensor_scalar(
                out=j2,
                in0=ps2,
                scalar1=1.0,
                scalar2=0.0,
                op0=mybir.AluOpType.mult,
                op1=mybir.AluOpType.add,
                accum_out=st[0:KPC, 1:2],
            )

            nc.sync.dma_start(out=stats[:, :], in_=st)

    _NC_CACHE = nc
    return nc


def _enable_jax_compile_cache():
    try:
        import jax

        jax.config.update("jax_compilation_cache_dir", "/tmp/jax_neff_cache")
        jax.config.update("jax_persistent_cache_min_entry_size_bytes", -1)
        jax.config.update("jax_persistent_cache_min_compile_time_secs", 0.0)
    except Exception:
        pass
    # NEFF disk cache keyed on BIR bytes (deterministic serialization):
    # skip walrus recompiles across processes.
    try:
        import hashlib
        import shutil

        from concourse import bass2jax

        orig = bass2jax.compile_bir_kernel
        if getattr(orig, "_neff_cache_wrapped", False):
            return

        def cached_compile(bir_json, tmpdir, neff_name="file.neff"):
            h = hashlib.sha256(
                bir_json if isinstance(bir_json, bytes) else bir_json.encode()
            ).hexdigest()
            cpath = f"/tmp/neff_cache/{h}.neff"
            if os.path.exists(cpath):
                dst = os.path.join(tmpdir, neff_name)
                shutil.copy(cpath, dst)
                return dst
            out = orig(bir_json, tmpdir, neff_name=neff_name)
            os.makedirs("/tmp/neff_cache", exist_ok=True)
            shutil.copy(out, cpath)
            return out

        cached_compile._neff_cache_wrapped = True
        bass2jax.compile_bir_kernel = cached_compile
    except Exception:
        pass


def kernel(pred_emb, gt_objmask, gt_classes):
    global LAST_RESULT
    pred_emb = np.asarray(pred_emb)
    gt_objmask = np.asarray(gt_objmask)
    cls = np.clip(np.asarray(gt_classes).astype(np.int64), 0, C - 1)
    k = gt_objmask.shape[0]

    _enable_jax_compile_cache()
    nc = _build_program()
    if not getattr(nc, "_sync_split_done", False):
        _split_sync(nc)  # CoreSim can't execute the bare NoOps; HW path only
        nc._sync_split_done = True

    f8 = mybir.dt.np(mybir.dt.float8e4)
    f16 = np.float16
    emb8 = pred_emb.astype(f8).reshape(C, P, F)
    maskff = (gt_objmask.astype(np.uint8) * np.uint8(0xFF)).reshape(k, P, F)
    cnt = np.count_nonzero(gt_objmask.reshape(k, -1), axis=1).astype(np.float64)

    sel8 = np.zeros((P, 26), dtype=f8)
    sel8[:, 13] = 1.0
    sel16 = np.zeros((P, 26), dtype=f16)
    sel16[:, 13] = 1.0

    in_maps = []
    for c in range(N_CORES):
        lo, hi = c * KPC, min((c + 1) * KPC, k)
        n = max(hi - lo, 0)
        pl = np.zeros((P, KPC, F), dtype=np.uint8)
        mk = np.zeros((P, KPC, F), dtype=np.uint8)
        if n > 0:
            pl[:, :n] = emb8[cls[lo:hi]].transpose(1, 0, 2).view(np.uint8)
            mk[:, :n] = maskff[lo:hi].transpose(1, 0, 2)
        in_maps.append(
            {
                "planes": pl.view(np.uint32),
                "masks": mk.view(np.uint32),
                "sel8": sel8,
                "sel16": sel16,
            }
        )

    core_ids = list(range(N_CORES))
    trace = bool(os.environ.get("KERNEL_TRACE"))
    res = run_bass_kernel_spmd(
        nc,
        in_maps,
        core_ids,
        trace=trace,
        trace_cores=core_ids if trace else None,
    )
    LAST_RESULT = res

    s1 = np.zeros(k, dtype=np.float64)
    s2 = np.zeros(k, dtype=np.float64)
    for c in range(N_CORES):
        lo, hi = c * KPC, min((c + 1) * KPC, k)
        n = max(hi - lo, 0)
        if n == 0:
            continue
        stats = res.results[c]["stats"].astype(np.float64)  # (P, 2)
        s1[lo:hi] = stats[:n, 0]
        s2[lo:hi] = stats[:n, 1]

    has = cnt > 0
    safe = np.where(has, cnt, 1.0)
    mean = np.where(has, s1 / safe, 0.0)
    var = np.where(has, s2 / safe - mean * mean, 0.0)

    same = cls[:, None] == cls[None, :]
    upper = np.triu(np.ones((k, k), dtype=bool), 1)
    diff2 = (mean[:, None] - mean[None, :]) ** 2
    hinge = np.maximum(1.0 - diff2, 0.0)
    loss_inter = np.sum(np.where(same & upper, hinge, 0.0))
    loss_reg = np.mean(mean * mean)
    loss_intra = np.mean(var)
    loss = 1.0 * loss_inter + 1.0 * loss_reg + 1.0 * loss_intra
    return np.array([loss], dtype=np.float32)


# revision 9
# speedup vs baseline: 1.3615x; 1.2260x over previous
"""Trainium2 Bass kernel for nn_Embedding_loss (masked per-instance embedding loss).

Math: for each instance k with class c_k, over the (H,W) plane:
    cnt_k = sum(mask_k), s1_k = sum(emb[c_k] * mask_k), s2_k = sum(emb[c_k]^2 * mask_k)
With m1 = emb * mask and mask in {0,1}:  s2_k = sum(m1^2).
Per-instance means/variances plus the tiny O(K^2) pairwise hinge term are
assembled on the host from the (s1, s2, cnt) triples.

Sharding: K instances are split across 8 cores (13 per core, zero-padded).

Device decomposition (per core) — avoids the two 1x-mode walls (fp8 elementwise
and accum_out ops both run at 1 elem/cycle on VectorE):
  - masks staged as 0x00/0xFF bytes; m1 = plane AND mask done on VectorE at
    uint32 granularity (4 fp8 bytes/lane/cycle, exact) in one batched op/wave.
  - squares: one batched ScalarE Square activation per wave (fp8 -> fp16).
  - per-instance sums s1, s2: TensorE matmuls with a ones[128,1] stationary;
    each instance accumulates its 4 chunk-column-sums into its own PSUM row.
    Two VectorE passes (one per PSUM bank) drain all 13 s1 and 13 s2 with a
    single accumulate each.
"""

import os

import numpy as np

import concourse.bass as bass
import concourse.tile as tile
from concourse import mybir
from concourse.bass_utils import run_bass_kernel_spmd

N_CORES = 8
C, H, W = 80, 512, 512
K = 100
KPC = 13  # instances per core (8*13 = 104 >= 100, padded with zero masks)
P = 128  # SBUF partitions
F = (H * W) // P  # free-dim elements per partition (2048)
FW = F // 4  # uint32 words per partition per instance (512)

WAVES = [2, 4, 4, 3]  # instances per DMA wave
DVE_SQ = (2, 6, 10)  # first instance of waves 1-3: squared on VectorE w/ accum
M1_OFF = 196608  # fixed SBUF byte offset of the dual-dtype m1 region

_NC_CACHE = None
LAST_RESULT = None  # BassKernelResults of the most recent run (for test harness)


def _split_sync(nc, max_w=1, max_u=1):
    """Walrus in this env accepts at most one sync wait/update per instruction;
    Tile's kernel-tail drain aggregates several. Split extras onto NoOps on the
    same engine (sequential waits on one queue are an AND, so semantics hold)."""
    ctr = 0
    for f in nc.m.functions:
        for bb in f.blocks:
            new = []
            for inst in bb.instructions:
                si = getattr(inst, "sync_info", None)
                waits = list(si.on_wait) if si is not None and si.on_wait else []
                updates = (
                    list(si.on_update) if si is not None and si.on_update else []
                )
                pre, post = [], []
                if len(waits) > max_w:
                    extra, keep = waits[:-max_w], waits[-max_w:]
                    si.on_wait = keep
                    for w in extra:
                        ctr += 1
                        nop = mybir.InstNoOp(name=f"syncsplit-w-{ctr}", ins=[], outs=[])
                        nop.engine = inst.engine
                        nop.sync_info = mybir.SyncInfo(on_wait=[w], on_update=[])
                        pre.append(nop)
                if len(updates) > max_u:
                    keep_u, extra_u = updates[:max_u], updates[max_u:]
                    si.on_update = keep_u
                    for u in extra_u:
                        ctr += 1
                        nop = mybir.InstNoOp(name=f"syncsplit-u-{ctr}", ins=[], outs=[])
                        nop.engine = inst.engine
                        nop.sync_info = mybir.SyncInfo(on_wait=[], on_update=[u])
                        post.append(nop)
                new.extend(pre)
                new.append(inst)
                new.extend(post)
            bb.instructions = new


def _build_program():
    """One SPMD Bass program: stream KPC (plane, mask) pairs, emit (s1, s2)."""
    global _NC_CACHE
    if _NC_CACHE is not None:
        return _NC_CACHE

    nc = bass.Bass()
    planes = nc.declare_dram_parameter(
        "planes", [P, KPC, FW], mybir.dt.uint32, isOutput=False
    )
    masks = nc.declare_dram_parameter(
        "masks", [P, KPC, FW], mybir.dt.uint32, isOutput=False
    )
    # sliding pair-one-hot for DoubleRow matmuls: cols 26,27 are 1.0; the
    # window sel[:, 26-2j : 52-2j] puts the ones-pair at relative cols (2j, 2j+1),
    # steering instance j's sums into PSUM row j (out partitions = 26/2 = 13).
    sel8 = nc.declare_dram_parameter(
        "sel8", [P, 2, 32], mybir.dt.float8e4, isOutput=False
    )
    # stats: col 0 = s1 (rows 0..KPC = instances), col 1 = s2 (PE instances),
    # cols 2.. = per-partition s2 partials for the DVE_SQ instances
    stats = nc.declare_dram_parameter("stats", [P, 8], mybir.dt.float32, isOutput=True)

    # m1 region, aliased as uint32 (bitwise-AND dest) and fp8 (Square/matmul src)
    m1u = nc.alloc_sbuf_tensor_at(
        "m1u", [P, KPC, FW], mybir.dt.uint32, offset=M1_OFF
    )
    m1f = nc.alloc_sbuf_tensor_at(
        "m1f", [P, KPC, 2, F // 2], mybir.dt.float8e4, offset=M1_OFF
    )

    waves = []
    lo = 0
    for w in WAVES:
        waves.append((lo, lo + w))
        lo += w
    assert lo == KPC

    with tile.TileContext(nc) as tc:
        with (
            tc.tile_pool(name="io", bufs=2) as io,
            tc.tile_pool(name="sqp", bufs=1) as sqp,
            tc.tile_pool(name="onesp", bufs=1) as onesp,
            tc.tile_pool(name="junkp", bufs=2) as junkp,
            tc.tile_pool(name="statp", bufs=1) as statp,
            tc.tile_pool(name="ps", bufs=1, space="PSUM") as ps,
        ):
            o8 = onesp.tile([P, 2, 32], mybir.dt.float8e4, tag="o8")
            nc.sync.dma_start(out=o8, in_=sel8[:, :, :])

            st = statp.tile([P, 8], mybir.dt.float32)
            sq = sqp.tile([P, KPC, 2, F // 2], mybir.dt.float8e4)
            ps1 = ps.tile([KPC, FW], mybir.dt.float32, tag="ps1")
            ps2 = ps.tile([KPC, FW], mybir.dt.float32, tag="ps2")

            for lo, hi in waves:
                n = hi - lo
                pt = io.tile([P, max(WAVES), FW], mybir.dt.uint32, tag="p")
                mt = io.tile([P, max(WAVES), FW], mybir.dt.uint32, tag="m")
                nc.sync.dma_start(out=pt[:, :n, :], in_=planes[:, lo:hi, :])
                nc.scalar.dma_start(out=mt[:, :n, :], in_=masks[:, lo:hi, :])

                # batched masked-plane: m1 = plane AND maskFF (exact, 4B/cycle)
                nc.vector.tensor_tensor(
                    out=m1u[:, lo:hi, :],
                    in0=pt[:, :n, :],
                    in1=mt[:, :n, :],
                    op=mybir.AluOpType.bitwise_and,
                )
                # squares for the DVE_SQ instance of this wave: fp8 STT with
                # fused accumulation (s2 lands as per-partition partials)
                alo = lo
                if lo > 0 and lo in DVE_SQ:
                    i = lo - lo
                    jk = junkp.tile([P, F], mybir.dt.float16, tag="jsq")
                    nc.vector.scalar_tensor_tensor(
                        out=jk,
                        in0=m1f[:, lo, :, :],
                        scalar=1.0,
                        in1=m1f[:, lo, :, :],
                        op0=mybir.AluOpType.mult,
                        op1=mybir.AluOpType.mult,
                        accum_out=st[:, 2 + DVE_SQ.index(lo) : 3 + DVE_SQ.index(lo)],
                    )
                    alo = lo + 1
                # batched squares on ScalarE (fp8 in, fp8 out) for the rest
                nc.scalar.activation(
                    out=sq[:, alo:hi, :, :],
                    in_=m1f[:, alo:hi, :, :],
                    func=mybir.ActivationFunctionType.Square,
                )
                # per-instance sums into PSUM rows: DoubleRow fp8 matmuls
                # (reduction tile 2 -> rhs spans 1024 cols, out 512)
                pe_set = [j for j in range(KPC) if j not in DVE_SQ]
                for j in range(lo, hi):
                    for c in range(2):
                        nc.tensor.matmul(
                            ps1[:, :],
                            o8[:, :, 13 - j : 26 - j],
                            m1f[:, j, :, c * FW : (c + 1) * FW],
                            start=(j == 0 and c == 0),
                            stop=(j == KPC - 1 and c == 1),
                            perf_mode=mybir.MatmulPerfMode.DoubleRow,
                            skip_group_check=True,
                        )
                    if j in DVE_SQ:
                        continue
                    for c in range(2):
                        nc.tensor.matmul(
                            ps2[:, :],
                            o8[:, :, 13 - j : 26 - j],
                            sq[:, j, :, c * FW : (c + 1) * FW],
                            start=(j == pe_set[0] and c == 0),
                            stop=(j == pe_set[-1] and c == 1),
                            perf_mode=mybir.MatmulPerfMode.DoubleRow,
                            skip_group_check=True,
                        )

            # drain: one accumulate per PSUM bank recovers all KPC sums
            j1 = junkp.tile([KPC, FW], mybir.dt.float32, tag="j1")
            j2 = junkp.tile([KPC, FW], mybir.dt.float32, tag="j2")
            nc.vector.tensor_scalar(
                out=j1,
                in0=ps1,
                scalar1=1.0,
                scalar2=0.0,
                op0=mybir.AluOpType.mult,
                op1=mybir.AluOpType.add,
                accum_out=st[0:KPC, 0:1],
            )
            nc.vector.tensor_scalar(
                out=j2,
                in0=ps2,
                scalar1=1.0,
                scalar2=0.0,
                op0=mybir.AluOpType.mult,
                op1=mybir.AluOpType.add,
                accum_out=st[0:KPC, 1:2],
            )

            nc.sync.dma_start(out=stats[:, :], in_=st)

    _NC_CACHE = nc
    return nc


def _enable_jax_compile_cache():
    try:
        import jax

        jax.config.update("jax_compilation_cache_dir", "/tmp/jax_neff_cache")
        jax.config.update("jax_persistent_cache_min_entry_size_bytes", -1)
        jax.config.update("jax_persistent_cache_min_compile_time_secs", 0.0)
    except Exception:
        pass
    # NEFF disk cache keyed on BIR bytes (deterministic serialization):
    # skip walrus recompiles across processes.
    try:
        import hashlib
        import shutil

        from concourse import bass2jax

        orig = bass2jax.compile_bir_kernel
        if getattr(orig, "_neff_cache_wrapped", False):
            return

        def cached_compile(bir_json, tmpdir, neff_name="file.neff"):
            h = hashlib.sha256(
                bir_json if isinstance(bir_json, bytes) else bir_json.encode()
            ).hexdigest()
            cpath = f"/tmp/neff_cache/{h}.neff"
            if os.path.exists(cpath):
                dst = os.path.join(tmpdir, neff_name)
                shutil.copy(cpath, dst)
                return dst
            out = orig(bir_json, tmpdir, neff_name=neff_name)
            os.makedirs("/tmp/neff_cache", exist_ok=True)
            shutil.copy(out, cpath)
            return out

        cached_compile._neff_cache_wrapped = True
        bass2jax.compile_bir_kernel = cached_compile
    except Exception:
        pass


def kernel(pred_emb, gt_objmask, gt_classes):
    global LAST_RESULT
    pred_emb = np.asarray(pred_emb)
    gt_objmask = np.asarray(gt_objmask)
    cls = np.clip(np.asarray(gt_classes).astype(np.int64), 0, C - 1)
    k = gt_objmask.shape[0]

    _enable_jax_compile_cache()
    nc = _build_program()
    if not getattr(nc, "_sync_split_done", False):
        _split_sync(nc)  # CoreSim can't execute the bare NoOps; HW path only
        nc._sync_split_done = True

    f8 = mybir.dt.np(mybir.dt.float8e4)
    emb8 = pred_emb.astype(f8).reshape(C, P, F)
    maskff = (gt_objmask.astype(np.uint8) * np.uint8(0xFF)).reshape(k, P, F)
    cnt = np.count_nonzero(gt_objmask.reshape(k, -1), axis=1).astype(np.float64)

    sel8 = np.zeros((P, 2, 32), dtype=f8)
    sel8[:, :, 13] = 1.0

    in_maps = []
    for c in range(N_CORES):
        lo, hi = c * KPC, min((c + 1) * KPC, k)
        n = max(hi - lo, 0)
        pl = np.zeros((P, KPC, F), dtype=np.uint8)
        mk = np.zeros((P, KPC, F), dtype=np.uint8)
        if n > 0:
            pl[:, :n] = emb8[cls[lo:hi]].transpose(1, 0, 2).view(np.uint8)
            mk[:, :n] = maskff[lo:hi].transpose(1, 0, 2)
        in_maps.append(
            {
                "planes": pl.view(np.uint32),
                "masks": mk.view(np.uint32),
                "sel8": sel8,
            }
        )

    core_ids = list(range(N_CORES))
    trace = bool(os.environ.get("KERNEL_TRACE"))
    res = run_bass_kernel_spmd(
        nc,
        in_maps,
        core_ids,
        trace=trace,
        trace_cores=core_ids if trace else None,
    )
    LAST_RESULT = res

    s1 = np.zeros(k, dtype=np.float64)
    s2 = np.zeros(k, dtype=np.float64)
    for c in range(N_CORES):
        lo, hi = c * KPC, min((c + 1) * KPC, k)
        n = max(hi - lo, 0)
        if n == 0:
            continue
        stats = res.results[c]["stats"].astype(np.float64)  # (P, 8)
        s1[lo:hi] = stats[:n, 0]
        s2[lo:hi] = stats[:n, 1]
        for idx, j in enumerate(DVE_SQ):
            if j < n:
                s2[lo + j] = stats[:, 2 + idx].sum()

    has = cnt > 0
    safe = np.where(has, cnt, 1.0)
    mean = np.where(has, s1 / safe, 0.0)
    var = np.where(has, s2 / safe - mean * mean, 0.0)

    same = cls[:, None] == cls[None, :]
    upper = np.triu(np.ones((k, k), dtype=bool), 1)
    diff2 = (mean[:, None] - mean[None, :]) ** 2
    hinge = np.maximum(1.0 - diff2, 0.0)
    loss_inter = np.sum(np.where(same & upper, hinge, 0.0))
    loss_reg = np.mean(mean * mean)
    loss_intra = np.mean(var)
    loss = 1.0 * loss_inter + 1.0 * loss_reg + 1.0 * loss_intra
    return np.array([loss], dtype=np.float32)


# revision 10
# speedup vs baseline: 1.3656x; 1.0030x over previous
"""Trainium2 Bass kernel for nn_Embedding_loss (masked per-instance embedding loss).

Math: for each instance k with class c_k, over the (H,W) plane:
    cnt_k = sum(mask_k), s1_k = sum(emb[c_k] * mask_k), s2_k = sum(emb[c_k]^2 * mask_k)
With m1 = emb * mask and mask in {0,1}:  s2_k = sum(m1^2).
Per-instance means/variances plus the tiny O(K^2) pairwise hinge term are
assembled on the host from the (s1, s2, cnt) triples.

Sharding: K instances are split across 8 cores (13 per core, zero-padded).

Device decomposition (per core) — avoids the two 1x-mode walls (fp8 elementwise
and accum_out ops both run at 1 elem/cycle on VectorE):
  - masks staged as 0x00/0xFF bytes; m1 = plane AND mask done on VectorE at
    uint32 granularity (4 fp8 bytes/lane/cycle, exact) in one batched op/wave.
  - squares: one batched ScalarE Square activation per wave (fp8 -> fp16).
  - per-instance sums s1, s2: TensorE matmuls with a ones[128,1] stationary;
    each instance accumulates its 4 chunk-column-sums into its own PSUM row.
    Two VectorE passes (one per PSUM bank) drain all 13 s1 and 13 s2 with a
    single accumulate each.
"""

import os

import numpy as np

import concourse.bass as bass
import concourse.tile as tile
from concourse import mybir
from concourse.bass_utils import run_bass_kernel_spmd

N_CORES = 8
C, H, W = 80, 512, 512
K = 100
KPC = 13  # instances per core (8*13 = 104 >= 100, padded with zero masks)
P = 128  # SBUF partitions
F = (H * W) // P  # free-dim elements per partition (2048)
FW = F // 4  # uint32 words per partition per instance (512)

WAVES = [1, 4, 4, 4]  # instances per DMA wave
DVE_SQ = (1, 2, 5, 6, 9)  # wave-prefix instances squared on VectorE w/ accum
M1_OFF = 196608  # fixed SBUF byte offset of the dual-dtype m1 region

_NC_CACHE = None
LAST_RESULT = None  # BassKernelResults of the most recent run (for test harness)


def _split_sync(nc, max_w=1, max_u=1):
    """Walrus in this env accepts at most one sync wait/update per instruction;
    Tile's kernel-tail drain aggregates several. Split extras onto NoOps on the
    same engine (sequential waits on one queue are an AND, so semantics hold)."""
    ctr = 0
    for f in nc.m.functions:
        for bb in f.blocks:
            new = []
            for inst in bb.instructions:
                si = getattr(inst, "sync_info", None)
                waits = list(si.on_wait) if si is not None and si.on_wait else []
                updates = (
                    list(si.on_update) if si is not None and si.on_update else []
                )
                pre, post = [], []
                if len(waits) > max_w:
                    extra, keep = waits[:-max_w], waits[-max_w:]
                    si.on_wait = keep
                    for w in extra:
                        ctr += 1
                        nop = mybir.InstNoOp(name=f"syncsplit-w-{ctr}", ins=[], outs=[])
                        nop.engine = inst.engine
                        nop.sync_info = mybir.SyncInfo(on_wait=[w], on_update=[])
                        pre.append(nop)
                if len(updates) > max_u:
                    keep_u, extra_u = updates[:max_u], updates[max_u:]
                    si.on_update = keep_u
                    for u in extra_u:
                        ctr += 1
                        nop = mybir.InstNoOp(name=f"syncsplit-u-{ctr}", ins=[], outs=[])
                        nop.engine = inst.engine
                        nop.sync_info = mybir.SyncInfo(on_wait=[], on_update=[u])
                        post.append(nop)
                new.extend(pre)
                new.append(inst)
                new.extend(post)
            bb.instructions = new


def _build_program():
    """One SPMD Bass program: stream KPC (plane, mask) pairs, emit (s1, s2)."""
    global _NC_CACHE
    if _NC_CACHE is not None:
        return _NC_CACHE

    nc = bass.Bass()
    planes = nc.declare_dram_parameter(
        "planes", [P, KPC, FW], mybir.dt.uint32, isOutput=False
    )
    masks = nc.declare_dram_parameter(
        "masks", [P, KPC, FW], mybir.dt.uint32, isOutput=False
    )
    # sliding pair-one-hot for DoubleRow matmuls: cols 26,27 are 1.0; the
    # window sel[:, 26-2j : 52-2j] puts the ones-pair at relative cols (2j, 2j+1),
    # steering instance j's sums into PSUM row j (out partitions = 26/2 = 13).
    sel8 = nc.declare_dram_parameter(
        "sel8", [P, 2, 32], mybir.dt.float8e4, isOutput=False
    )
    # stats: col 0 = s1 (rows 0..KPC = instances), col 1 = s2 (PE instances),
    # cols 2.. = per-partition s2 partials for the DVE_SQ instances
    stats = nc.declare_dram_parameter("stats", [P, 8], mybir.dt.float32, isOutput=True)

    # m1 region, aliased as uint32 (bitwise-AND dest) and fp8 (Square/matmul src)
    m1u = nc.alloc_sbuf_tensor_at(
        "m1u", [P, KPC, FW], mybir.dt.uint32, offset=M1_OFF
    )
    m1f = nc.alloc_sbuf_tensor_at(
        "m1f", [P, KPC, 2, F // 2], mybir.dt.float8e4, offset=M1_OFF
    )

    waves = []
    lo = 0
    for w in WAVES:
        waves.append((lo, lo + w))
        lo += w
    assert lo == KPC

    with tile.TileContext(nc) as tc:
        with (
            tc.tile_pool(name="io", bufs=2) as io,
            tc.tile_pool(name="sqp", bufs=1) as sqp,
            tc.tile_pool(name="onesp", bufs=1) as onesp,
            tc.tile_pool(name="junkp", bufs=2) as junkp,
            tc.tile_pool(name="statp", bufs=1) as statp,
            tc.tile_pool(name="ps", bufs=1, space="PSUM") as ps,
        ):
            o8 = onesp.tile([P, 2, 32], mybir.dt.float8e4, tag="o8")

            st = statp.tile([P, 8], mybir.dt.float32)
            sq = sqp.tile([P, KPC, 2, F // 2], mybir.dt.float8e4)
            ps1 = ps.tile([KPC, FW], mybir.dt.float32, tag="ps1")
            ps2 = ps.tile([KPC, FW], mybir.dt.float32, tag="ps2")

            for lo, hi in waves:
                n = hi - lo
                pt = io.tile([P, max(WAVES), FW], mybir.dt.uint32, tag="p")
                mt = io.tile([P, max(WAVES), FW], mybir.dt.uint32, tag="m")
                nc.sync.dma_start(out=pt[:, :n, :], in_=planes[:, lo:hi, :])
                nc.scalar.dma_start(out=mt[:, :n, :], in_=masks[:, lo:hi, :])
                if lo == 0:
                    nc.sync.dma_start(out=o8, in_=sel8[:, :, :])

                # batched masked-plane: m1 = plane AND maskFF (exact, 4B/cycle)
                nc.vector.tensor_tensor(
                    out=m1u[:, lo:hi, :],
                    in0=pt[:, :n, :],
                    in1=mt[:, :n, :],
                    op=mybir.AluOpType.bitwise_and,
                )
                # squares for this wave's DVE_SQ prefix: fp8 STT with fused
                # accumulation (s2 lands as per-partition partials)
                alo = lo
                while alo < hi and alo in DVE_SQ:
                    jk = junkp.tile([P, F], mybir.dt.float16, tag="jsq")
                    nc.vector.scalar_tensor_tensor(
                        out=jk,
                        in0=m1f[:, alo, :, :],
                        scalar=1.0,
                        in1=m1f[:, alo, :, :],
                        op0=mybir.AluOpType.mult,
                        op1=mybir.AluOpType.mult,
                        accum_out=st[:, 2 + DVE_SQ.index(alo) : 3 + DVE_SQ.index(alo)],
                    )
                    alo += 1
                # batched squares on ScalarE (fp8 in, fp8 out) for the rest
                nc.scalar.activation(
                    out=sq[:, alo:hi, :, :],
                    in_=m1f[:, alo:hi, :, :],
                    func=mybir.ActivationFunctionType.Square,
                )
                # per-instance sums into PSUM rows: DoubleRow fp8 matmuls
                # (reduction tile 2 -> rhs spans 1024 cols, out 512)
                pe_set = [j for j in range(KPC) if j not in DVE_SQ]
                for j in range(lo, hi):
                    for c in range(2):
                        nc.tensor.matmul(
                            ps1[:, :],
                            o8[:, :, 13 - j : 26 - j],
                            m1f[:, j, :, c * FW : (c + 1) * FW],
                            start=(j == 0 and c == 0),
                            stop=(j == KPC - 1 and c == 1),
                            perf_mode=mybir.MatmulPerfMode.DoubleRow,
                            skip_group_check=True,
                        )
                    if j in DVE_SQ:
                        continue
                    for c in range(2):
                        nc.tensor.matmul(
                            ps2[:, :],
                            o8[:, :, 13 - j : 26 - j],
                            sq[:, j, :, c * FW : (c + 1) * FW],
                            start=(j == pe_set[0] and c == 0),
                            stop=(j == pe_set[-1] and c == 1),
                            perf_mode=mybir.MatmulPerfMode.DoubleRow,
                            skip_group_check=True,
                        )

            # drain: one accumulate per PSUM bank recovers all KPC sums
            j1 = junkp.tile([KPC, FW], mybir.dt.float32, tag="j1")
            j2 = junkp.tile([KPC, FW], mybir.dt.float32, tag="j2")
            nc.vector.tensor_scalar(
                out=j1,
                in0=ps1,
                scalar1=1.0,
                scalar2=0.0,
                op0=mybir.AluOpType.mult,
                op1=mybir.AluOpType.add,
                accum_out=st[0:KPC, 0:1],
            )
            nc.vector.tensor_scalar(
                out=j2,
                in0=ps2,
                scalar1=1.0,
                scalar2=0.0,
                op0=mybir.AluOpType.mult,
                op1=mybir.AluOpType.add,
                accum_out=st[0:KPC, 1:2],
            )

            nc.sync.dma_start(out=stats[:, :], in_=st)

    _NC_CACHE = nc
    return nc


def _enable_jax_compile_cache():
    try:
        import jax

        jax.config.update("jax_compilation_cache_dir", "/tmp/jax_neff_cache")
        jax.config.update("jax_persistent_cache_min_entry_size_bytes", -1)
        jax.config.update("jax_persistent_cache_min_compile_time_secs", 0.0)
    except Exception:
        pass
    # NEFF disk cache keyed on BIR bytes (deterministic serialization):
    # skip walrus recompiles across processes.
    try:
        import hashlib
        import shutil

        from concourse import bass2jax

        orig = bass2jax.compile_bir_kernel
        if getattr(orig, "_neff_cache_wrapped", False):
            return

        def cached_compile(bir_json, tmpdir, neff_name="file.neff"):
            h = hashlib.sha256(
                bir_json if isinstance(bir_json, bytes) else bir_json.encode()
            ).hexdigest()
            cpath = f"/tmp/neff_cache/{h}.neff"
            if os.path.exists(cpath):
                dst = os.path.join(tmpdir, neff_name)
                shutil.copy(cpath, dst)
                return dst
            out = orig(bir_json, tmpdir, neff_name=neff_name)
            os.makedirs("/tmp/neff_cache", exist_ok=True)
            shutil.copy(out, cpath)
            return out

        cached_compile._neff_cache_wrapped = True
        bass2jax.compile_bir_kernel = cached_compile
    except Exception:
        pass


def kernel(pred_emb, gt_objmask, gt_classes):
    global LAST_RESULT
    pred_emb = np.asarray(pred_emb)
    gt_objmask = np.asarray(gt_objmask)
    cls = np.clip(np.asarray(gt_classes).astype(np.int64), 0, C - 1)
    k = gt_objmask.shape[0]

    _enable_jax_compile_cache()
    nc = _build_program()
    if not getattr(nc, "_sync_split_done", False):
        _split_sync(nc)  # CoreSim can't execute the bare NoOps; HW path only
        nc._sync_split_done = True

    f8 = mybir.dt.np(mybir.dt.float8e4)
    emb8 = pred_emb.astype(f8).reshape(C, P, F)
    maskff = (gt_objmask.astype(np.uint8) * np.uint8(0xFF)).reshape(k, P, F)
    cnt = np.count_nonzero(gt_objmask.reshape(k, -1), axis=1).astype(np.float64)

    sel8 = np.zeros((P, 2, 32), dtype=f8)
    sel8[:, :, 13] = 1.0

    in_maps = []
    for c in range(N_CORES):
        lo, hi = c * KPC, min((c + 1) * KPC, k)
        n = max(hi - lo, 0)
        pl = np.zeros((P, KPC, F), dtype=np.uint8)
        mk = np.zeros((P, KPC, F), dtype=np.uint8)
        if n > 0:
            pl[:, :n] = emb8[cls[lo:hi]].transpose(1, 0, 2).view(np.uint8)
            mk[:, :n] = maskff[lo:hi].transpose(1, 0, 2)
        in_maps.append(
            {
                "planes": pl.view(np.uint32),
                "masks": mk.view(np.uint32),
                "sel8": sel8,
            }
        )

    core_ids = list(range(N_CORES))
    trace = bool(os.environ.get("KERNEL_TRACE"))
    res = run_bass_kernel_spmd(
        nc,
        in_maps,
        core_ids,
        trace=trace,
        trace_cores=core_ids if trace else None,
    )
    LAST_RESULT = res

    s1 = np.zeros(k, dtype=np.float64)
    s2 = np.zeros(k, dtype=np.float64)
    for c in range(N_CORES):
        lo, hi = c * KPC, min((c + 1) * KPC, k)
        n = max(hi - lo, 0)
        if n == 0:
            continue
        stats = res.results[c]["stats"].astype(np.float64)  # (P, 8)
        s1[lo:hi] = stats[:n, 0]
        s2[lo:hi] = stats[:n, 1]
        for idx, j in enumerate(DVE_SQ):
            if j < n:
                s2[lo + j] = stats[:, 2 + idx].sum()

    has = cnt > 0
    safe = np.where(has, cnt, 1.0)
    mean = np.where(has, s1 / safe, 0.0)
    var = np.where(has, s2 / safe - mean * mean, 0.0)

    same = cls[:, None] == cls[None, :]
    upper = np.triu(np.ones((k, k), dtype=bool), 1)
    diff2 = (mean[:, None] - mean[None, :]) ** 2
    hinge = np.maximum(1.0 - diff2, 0.0)
    loss_inter = np.sum(np.where(same & upper, hinge, 0.0))
    loss_reg = np.mean(mean * mean)
    loss_intra = np.mean(var)
    loss = 1.0 * loss_inter + 1.0 * loss_reg + 1.0 * loss_intra
    return np.array([loss], dtype=np.float32)
